# revision 1
# baseline (speedup 1.0000x reference)
"""Chamfer loss kernel for Trainium2 (8 NeuronCores, batch-data-parallel).

Math: for each batch b, dist_sq[n,m] = |p3[n]|^2 + |q3[m]|^2 - 2 p3[n].q3[m].
The reference takes sqrt(max(dist_sq,0)+eps) then dual-axis mins then sums.
sqrt/max/+eps are monotone, so min commutes with them: the device only
computes min_m dist_sq (per n) and min_n dist_sq (per m); the host applies
sqrt(max(.,0)+eps) to the 2*B*N mins and sums in float64.

Device strategy (per core, 16 batches; two transposed passes give the two
reduction directions as plain row-mins):
  - One K=24 bf16 matmul per (batch, 128-row tile, 512-col bank) writes
    PSUM = dist_sq/2 directly. K rows: a 3-level bf16 split (h+l+r) of the
    3-vectors with pairings hh+hl+lh+hr+rh+ll (~1e-6 abs error, needed
    because fp32 matmul is 4x slower on the PE), plus 3-term splits of the
    point norms against `ones` rows so the row/col norm broadcasts ride in
    the same matmul.
  - 4 batches are packed into the 4 PE row-groups (operands at partitions
    32g..32g+KROWS) so 4 matmuls run concurrently in the array.
  - Reduction (the real bottleneck; only DVE can reduce, only DVE/ACT can
    read PSUM, and everything PSUM-sourced runs at 1 elem/cycle/lane):
    rounds of 4 batches live in two 4-bank PSUM pair-tensors. Per round,
    ScalarE evacuates pair tensors to fp16 SBUF, DVE folds them with
    2x-mode fp16 tensor_tensor mins (2048->1024->512->256 cols) and one
    small 1x reduce. Every DIRECT_EVERY-th round DVE instead reduces one
    pair straight from PSUM, balancing DVE vs ScalarE occupancy.
Output: (128, 256) fp32 per core = per-partition mins of dist_sq/2; host
decodes, finishes, and sums across cores.
"""

import numpy as np

import concourse.bass as bass  # noqa: F401  (bass types used via bacc/tile)
import concourse.mybir as mybir
import concourse.tile as tile
from concourse import bacc
from concourse.bass_utils import run_bass_kernel_spmd

B, N, M = 128, 1024, 1024
NCORES = 8
BPC = B // NCORES  # 16 batches per core
NQUAD = BPC // 4  # 4 quads of 4 batches
F32 = mybir.dt.float32
BF16 = mybir.dt.bfloat16
F16 = mybir.dt.float16
EVAC_DT = mybir.dt.float32  # dtype for ACT evacuation in hybrid4
KROWS = 24  # bf16 3-level split: 18 cross rows + 3 qn rows + 3 pn rows

_CACHE = {}
MODE = "hybrid5"  # production: fp16-evac folds + half-direct rounds
DIRECT_MOD = 4
DIRECT_EVERY = 5  # hybrid5: DIRECT_NUM of every DIRECT_EVERY rounds go half-direct
DIRECT_NUM = 3
# in hybrid2: item % DIRECT_MOD == 0 -> direct PSUM reduce


REPEAT = 1  # dev knob: repeat compute body (same output) for delta timing


def _body(tc, dram, res_d):
    nc = tc.nc
    with (
        tc.tile_pool(name="stacks", bufs=1) as stacks,
        tc.tile_pool(name="scratchp", bufs=1) as scratchp,
        tc.tile_pool(name="resp", bufs=1) as resp,
        tc.tile_pool(name="psump", bufs=1, space="PSUM") as psump,
    ):
        stk = {}
        # quad-0 slices of the lhsT/rhs stacks first so the first rounds'
        # matmuls are not gated on the whole 3 MB prologue transfer
        for nm in ("ap_s", "bq_s", "aq_s", "bp_s"):
            t = stacks.tile([128, NQUAD, 1024], BF16, name=nm + "_t", tag=nm + "_t")
            stk[nm] = t
        for part in range(2):
            for nm in ("ap_s", "bq_s", "aq_s", "bp_s"):
                t = stk[nm]
                for g in range(4):
                    if part == 0:
                        nc.sync.dma_start(
                            out=t[32 * g : 32 * g + KROWS, 0:1],
                            in_=dram[nm][g, :, 0:1],
                        )
                    else:
                        nc.sync.dma_start(
                            out=t[32 * g : 32 * g + KROWS, 1:NQUAD],
                            in_=dram[nm][g, :, 1:NQUAD],
                        )

        res_t = resp.tile([128, 2 * BPC * 8], F32, name="res_t", tag="res_t")
        scr = scratchp.tile([128, 512], F32, name="scr", tag="scr")

        passes = [(stk["ap_s"], stk["bq_s"]), (stk["aq_s"], stk["bp_s"])]

        def compute_hybrid5():
            # Evacuation rounds: ACT copies the round's two 4-bank pair
            # tensors to one fp16 scratch (128,4,2,512); DVE folds all 4
            # items with three 2x_1p fp16 TT-mins (2048->1024->512->256
            # cols) and one 1x reduce -> 4 result columns.  Every
            # DIRECT_EVERY-th round instead reduces straight from PSUM
            # (soaks DVE slack while ACT is the binding engine).
            rnd = 0
            for d_i, (A, Bs) in enumerate(passes):
                for t_i in range(NQUAD):
                    for i in range(8):
                        pr = [
                            psump.tile(
                                [128, 2, 2, 512], F32, name=f"pr{h}", tag=f"pr{h}"
                            )
                            for h in range(2)
                        ]
                        for j in range(2):
                            for g in range(4):
                                nc.tensor.matmul(
                                    pr[g // 2][:, g % 2, j, :],
                                    A[
                                        32 * g : 32 * g + KROWS,
                                        t_i,
                                        128 * i : 128 * (i + 1),
                                    ],
                                    Bs[
                                        32 * g : 32 * g + KROWS,
                                        t_i,
                                        512 * j : 512 * (j + 1),
                                    ],
                                    start=True,
                                    stop=True,
                                    tile_position=(32 * g, 0),
                                )
                        base = ((d_i * NQUAD + t_i) * 8 + i) * 4
                        if DIRECT_EVERY > 0 and (rnd * DIRECT_NUM) % DIRECT_EVERY < DIRECT_NUM:
                            # half-direct round: DVE takes pair 0 straight
                            # from PSUM; ACT only copies pair 1
                            nc.vector.tensor_reduce(
                                out=res_t[:, base : base + 2],
                                in_=pr[0][:, :, :, :],
                                axis=mybir.AxisListType.XY,
                                op=mybir.AluOpType.min,
                            )
                            s2 = scratchp.tile(
                                [128, 2, 2, 512], F16, name="s5b", tag="s5b", bufs=3
                            )
                            u2 = scratchp.tile(
                                [128, 2, 512], F16, name="u5b", tag="u5b", bufs=3
                            )
                            w2 = scratchp.tile(
                                [128, 2, 256], F16, name="w5b", tag="w5b", bufs=3
                            )
                            x2 = scratchp.tile(
                                [128, 2, 128], F16, name="x5b", tag="x5b", bufs=3
                            )
                            nc.scalar.copy(s2, pr[1][:, :, :, :])
                            nc.vector.tensor_tensor(
                                out=u2,
                                in0=s2[:, :, 0, :],
                                in1=s2[:, :, 1, :],
                                op=mybir.AluOpType.min,
                            )
                            nc.vector.tensor_tensor(
                                out=w2,
                                in0=u2[:, :, 0:256],
                                in1=u2[:, :, 256:512],
                                op=mybir.AluOpType.min,
                            )
                            nc.vector.tensor_tensor(
                                out=x2,
                                in0=w2[:, :, 0:128],
                                in1=w2[:, :, 128:256],
                                op=mybir.AluOpType.min,
                            )
                            nc.vector.tensor_reduce(
                                out=res_t[:, base + 2 : base + 4],
                                in_=x2,
                                axis=mybir.AxisListType.X,
                                op=mybir.AluOpType.min,
                            )
                        else:
                            s = scratchp.tile(
                                [128, 4, 2, 512], F16, name="s5", tag="s5", bufs=3
                            )
                            u = scratchp.tile(
                                [128, 4, 512], F16, name="u5", tag="u5", bufs=3
                            )
                            w = scratchp.tile(
                                [128, 4, 256], F16, name="w5", tag="w5", bufs=3
                            )
                            for h in range(2):
                                nc.scalar.copy(
                                    s[:, 2 * h : 2 * h + 2], pr[h][:, :, :, :]
                                )
                            nc.vector.tensor_tensor(
                                out=u,
                                in0=s[:, :, 0, :],
                                in1=s[:, :, 1, :],
                                op=mybir.AluOpType.min,
                            )
                            nc.vector.tensor_tensor(
                                out=w,
                                in0=u[:, :, 0:256],
                                in1=u[:, :, 256:512],
                                op=mybir.AluOpType.min,
                            )
                            x = scratchp.tile(
                                [128, 4, 128], F16, name="x5", tag="x5", bufs=3
                            )
                            nc.vector.tensor_tensor(
                                out=x,
                                in0=w[:, :, 0:128],
                                in1=w[:, :, 128:256],
                                op=mybir.AluOpType.min,
                            )
                            nc.vector.tensor_reduce(
                                out=res_t[:, base : base + 4],
                                in_=x,
                                axis=mybir.AxisListType.X,
                                op=mybir.AluOpType.min,
                            )
                        rnd += 1

        def compute_hybrid4():
            # psum held as two 4-bank pair-tensors; per 2-round period:
            #   even round: pair01 direct (one 2-col reduce), pair23 assisted
            #   odd round: both pairs assisted
            # assisted = ACT evacuates pair (FD=2048) to SBUF, DVE
            # tensor_scalar min/min-accum per item at 2x_2p.
            rnd = 0
            for d_i, (A, Bs) in enumerate(passes):
                for t_i in range(NQUAD):
                    for i in range(8):
                        pr = [
                            psump.tile(
                                [128, 2, 2, 512], F32, name=f"pr{h}", tag=f"pr{h}"
                            )
                            for h in range(2)
                        ]
                        for j in range(2):
                            for g in range(4):
                                nc.tensor.matmul(
                                    pr[g // 2][:, g % 2, j, :],
                                    A[
                                        32 * g : 32 * g + KROWS,
                                        t_i,
                                        128 * i : 128 * (i + 1),
                                    ],
                                    Bs[
                                        32 * g : 32 * g + KROWS,
                                        t_i,
                                        512 * j : 512 * (j + 1),
                                    ],
                                    start=True,
                                    stop=True,
                                    tile_position=(32 * g, 0),
                                )
                        base = ((d_i * NQUAD + t_i) * 8 + i) * 4
                        for h in range(2):
                            col = base + 2 * h
                            if h == 0 and rnd % 2 == 0:
                                nc.vector.tensor_reduce(
                                    out=res_t[:, col : col + 2],
                                    in_=pr[h][:, :, :, :],
                                    axis=mybir.AxisListType.XY,
                                    op=mybir.AluOpType.min,
                                )
                            else:
                                cp = scratchp.tile(
                                    [128, 2, 2, 512],
                                    EVAC_DT,
                                    name=f"cpp{h}",
                                    tag=f"cpp{h}",
                                    bufs=2,
                                )
                                sc = scratchp.tile(
                                    [128, 2, 2, 512],
                                    EVAC_DT,
                                    name=f"scp{h}",
                                    tag=f"scp{h}",
                                    bufs=2,
                                )
                                nc.scalar.copy(cp, pr[h][:, :, :, :])
                                for u in range(2):
                                    nc.vector.tensor_scalar(
                                        out=sc[:, u],
                                        in0=cp[:, u],
                                        scalar1=60000.0,
                                        scalar2=None,
                                        op0=mybir.AluOpType.min,
                                        op1=mybir.AluOpType.min,
                                        accum_out=res_t[:, col + u : col + u + 1],
                                    )
                        rnd += 1

        def compute_once():
            if MODE == "hybrid5":
                compute_hybrid5()
                return
            if MODE == "hybrid4":
                compute_hybrid4()
                return
            for d_i, (A, Bs) in enumerate(passes):
                for t_i in range(NQUAD):
                    for i in range(8):
                        ps = [
                            psump.tile([128, 2, 512], F32, name=f"ps{g}", tag=f"ps{g}")
                            for g in range(4)
                        ]
                        for j in range(2):
                            for g in range(4):
                                nc.tensor.matmul(
                                    ps[g][:, j, :],
                                    A[
                                        32 * g : 32 * g + KROWS,
                                        t_i,
                                        128 * i : 128 * (i + 1),
                                    ],
                                    Bs[
                                        32 * g : 32 * g + KROWS,
                                        t_i,
                                        512 * j : 512 * (j + 1),
                                    ],
                                    start=True,
                                    stop=True,
                                    tile_position=(32 * g, 0),
                                )
                        for g in range(4):
                            col = ((d_i * NQUAD + t_i) * 8 + i) * 4 + g
                            item = ((d_i * NQUAD + t_i) * 8 + i) * 4 + g
                            if MODE == "hybrid3":
                                if DIRECT_MOD > 0 and item % DIRECT_MOD == 0:
                                    nc.vector.tensor_reduce(
                                        out=res_t[:, col : col + 1],
                                        in_=ps[g][:, :, :],
                                        axis=mybir.AxisListType.XY,
                                        op=mybir.AluOpType.min,
                                    )
                                else:
                                    # ACT evacuates to fp16 SBUF; DVE
                                    # tensor_scalar (min w/ min-accum) hits
                                    # 4x_2p on fp16 SBUF = 4 elem/cycle.
                                    cph = scratchp.tile(
                                        [128, 2, 512],
                                        F16,
                                        name=f"cph{g}",
                                        tag=f"cph{g}",
                                        bufs=2,
                                    )
                                    sch = scratchp.tile(
                                        [128, 2, 512],
                                        F16,
                                        name=f"sch{g}",
                                        tag=f"sch{g}",
                                        bufs=2,
                                    )
                                    nc.scalar.copy(cph, ps[g][:, :, :])
                                    nc.vector.tensor_scalar(
                                        out=sch,
                                        in0=cph,
                                        scalar1=60000.0,
                                        scalar2=None,
                                        op0=mybir.AluOpType.min,
                                        op1=mybir.AluOpType.min,
                                        accum_out=res_t[:, col : col + 1],
                                    )
                            elif MODE == "hybrid2":
                                if DIRECT_MOD > 0 and item % DIRECT_MOD == 0:
                                    # direct: DVE reduce straight from PSUM (1x)
                                    nc.vector.tensor_reduce(
                                        out=res_t[:, col : col + 1],
                                        in_=ps[g][:, :, :],
                                        axis=mybir.AxisListType.XY,
                                        op=mybir.AluOpType.min,
                                    )
                                else:
                                    # ACT evacuates both banks to SBUF; DVE
                                    # tensor_scalar (min w/ min-accum) runs
                                    # at 2x_2p on fp32 SBUF = 2 elem/cycle.
                                    cp = scratchp.tile(
                                        [128, 2, 512],
                                        F32,
                                        name=f"cp{g}",
                                        tag=f"cp{g}",
                                        bufs=2,
                                    )
                                    sc2 = scratchp.tile(
                                        [128, 2, 512],
                                        F32,
                                        name=f"sc2{g}",
                                        tag=f"sc2{g}",
                                        bufs=2,
                                    )
                                    nc.scalar.copy(cp, ps[g][:, :, :])
                                    nc.vector.tensor_scalar(
                                        out=sc2,
                                        in0=cp,
                                        scalar1=3.0e38,
                                        scalar2=None,
                                        op0=mybir.AluOpType.min,
                                        op1=mybir.AluOpType.min,
                                        accum_out=res_t[:, col : col + 1],
                                    )
                            elif MODE == "hybrid":
                                if item % 3 == 0:
                                    # direct: DVE reduce straight from PSUM
                                    nc.vector.tensor_reduce(
                                        out=res_t[:, col : col + 1],
                                        in_=ps[g][:, :, :],
                                        axis=mybir.AxisListType.XY,
                                        op=mybir.AluOpType.min,
                                    )
                                else:
                                    # assisted: ACT evacuates both banks,
                                    # GpSimd folds them, DVE reduces 512
                                    cp0 = scratchp.tile(
                                        [128, 512],
                                        F32,
                                        name=f"cp0{g}",
                                        tag=f"cp0{g}",
                                        bufs=2,
                                    )
                                    cp1 = scratchp.tile(
                                        [128, 512],
                                        F32,
                                        name=f"cp1{g}",
                                        tag=f"cp1{g}",
                                        bufs=2,
                                    )
                                    u = scratchp.tile(
                                        [128, 512],
                                        F32,
                                        name=f"u{g}",
                                        tag=f"u{g}",
                                        bufs=2,
                                    )
                                    nc.scalar.copy(cp0, ps[g][:, 0, :])
                                    nc.scalar.copy(cp1, ps[g][:, 1, :])
                                    nc.gpsimd.tensor_tensor(
                                        out=u,
                                        in0=cp0,
                                        in1=cp1,
                                        op=mybir.AluOpType.min,
                                    )
                                    nc.vector.tensor_reduce(
                                        out=res_t[:, col : col + 1],
                                        in_=u,
                                        axis=mybir.AxisListType.X,
                                        op=mybir.AluOpType.min,
                                    )
                            elif MODE == "scan":
                                # DVE may read only ONE PSUM operand: ACT
                                # evacuates bank j=1 to SBUF; the scan then
                                # streams PSUM bank 0 + the SBUF copy at 2
                                # cols/cycle, keeping a running min; the
                                # stride-0 out AP leaves the final min in the
                                # result column.
                                cp = scratchp.tile(
                                    [128, 512], F32, name=f"cp{g}", tag=f"cp{g}", bufs=2
                                )
                                nc.scalar.copy(cp, ps[g][:, 1, :])
                                nc.vector.tensor_tensor_scan(
                                    out=res_t[:, col : col + 1].broadcast_to((128, 512)),
                                    data0=ps[g][:, 0, :],
                                    data1=cp,
                                    initial=3.0e38,
                                    op0=mybir.AluOpType.min,
                                    op1=mybir.AluOpType.min,
                                )
                            elif MODE == "ttr":
                                # DVE may read only ONE PSUM operand per
                                # instruction: ACT evacuates bank j=1 to SBUF,
                                # TTR then streams PSUM bank j=0 + the SBUF
                                # copy (2 cols/cycle).
                                cp = scratchp.tile(
                                    [128, 512], F32, name=f"cp{g}", tag=f"cp{g}", bufs=2
                                )
                                nc.scalar.copy(cp, ps[g][:, 1, :])
                                nc.vector.tensor_tensor_reduce(
                                    out=scr,
                                    in0=ps[g][:, 0, :],
                                    in1=cp,
                                    scale=1.0,
                                    scalar=3.0e38,
                                    op0=mybir.AluOpType.min,
                                    op1=mybir.AluOpType.min,
                                    accum_out=res_t[:, col : col + 1],
                                )
                            else:
                                nc.vector.tensor_reduce(
                                    out=res_t[:, col : col + 1],
                                    in_=ps[g][:, :, :],
                                    axis=mybir.AxisListType.XY,
                                    op=mybir.AluOpType.min,
                                )

        if REPEAT == 1:
            compute_once()
        else:
            with tc.For_i(0, REPEAT, 1):
                compute_once()

        nc.sync.dma_start(out=res_d, in_=res_t)


def _build_nc():
    if "nc" in _CACHE:
        return _CACHE["nc"]
    nc = bacc.Bacc(
        "TRN2", target_bir_lowering=False, debug=False, num_devices=NCORES
    )
    dram = {}
    for nm in ("ap_s", "bq_s", "aq_s", "bp_s"):
        dram[nm] = nc.dram_tensor(
            nm, (4, KROWS, NQUAD, 1024), BF16, kind="ExternalInput"
        ).ap()
    res_d = nc.dram_tensor("res", (128, 2 * BPC * 8), F32, kind="ExternalOutput").ap()
    with tile.TileContext(nc) as tc:
        _body(tc, dram, res_d)
    nc.compile()
    _CACHE["nc"] = nc
    return nc


def _split3(x):
    """Split fp32 into 3 bf16 terms (x ~= h + l + r, error ~2^-27 |x|)."""
    import ml_dtypes

    bf = ml_dtypes.bfloat16
    h = x.astype(bf)
    l = (x - h.astype(np.float32)).astype(bf)
    r = (x - h.astype(np.float32) - l.astype(np.float32)).astype(bf)
    return h, l, r


def _host_stacks(x3, xn, lhs):
    """x3: (BPC, 1024, 3), xn: (BPC, 1024) -> (4, KROWS, NQUAD, 1024) bf16.

    Layout [g, k, t, n]: batch 4*t + g lives in PE row-group g (SBUF
    partitions 32g+k). With s = -x3 for lhsT (s = x3 for rhs) and
    h/l/r the bf16 3-level split, the K pairing slots are
      cross (x3): lhsT [h h l h r l], rhs [h l h r h l]  (x3 comps each)
      norms: lhsT [1 1 1 h(xn/2) l r], rhs [h(yn/2) l r 1 1 1]
    so lhsT[k]*rhs[k] accumulates hh+hl+lh+hr+rh+ll cross terms plus the
    3-term norm halves -> PSUM = dist_sq/2 with ~1e-6 absolute error."""
    import ml_dtypes

    bf = ml_dtypes.bfloat16
    out = np.empty((NQUAD, 4, KROWS, 1024), bf)  # [t, g, k, n]
    sign = -1.0 if lhs else 1.0
    x3t = np.transpose(
        (sign * x3).reshape(NQUAD, 4, 1024, 3), (0, 1, 3, 2)
    )  # (t,g,3,n)
    h3, l3, r3 = _split3(x3t)
    hn, ln, rn = _split3((xn * 0.5).reshape(NQUAD, 4, 1024))
    one = np.asarray(1.0, bf)
    if lhs:
        cross = (h3, h3, l3, h3, r3, l3)
        norm = (one, one, one, hn, ln, rn)
    else:
        cross = (h3, l3, h3, r3, h3, l3)
        norm = (hn, ln, rn, one, one, one)
    for s in range(6):
        out[:, :, 3 * s : 3 * s + 3] = cross[s]
        out[:, :, 18 + s] = norm[s]
    return np.ascontiguousarray(np.transpose(out, (1, 2, 0, 3)))


def _run(p, q, trace=False, tmpdir=None):
    p = np.asarray(p)
    q = np.asarray(q)
    assert p.shape == (B, N, 4) and q.shape == (B, M, 4)
    p3 = np.ascontiguousarray(p[:, :, 1:], dtype=np.float32)
    q3 = np.ascontiguousarray(q[:, :, 1:], dtype=np.float32)
    pn = np.einsum("bnc,bnc->bn", p3, p3)
    qn = np.einsum("bmc,bmc->bm", q3, q3)

    in_maps = []
    for c in range(NCORES):
        sl = slice(BPC * c, BPC * (c + 1))
        in_maps.append(
            {
                "ap_s": _host_stacks(p3[sl], pn[sl], lhs=True),
                "bq_s": _host_stacks(q3[sl], qn[sl], lhs=False),
                "aq_s": _host_stacks(q3[sl], qn[sl], lhs=True),
                "bp_s": _host_stacks(p3[sl], pn[sl], lhs=False),
            }
        )

    nc = _build_nc()
    kw = {}
    if trace:
        kw = {"trace": True, "tmpdir": tmpdir}
    rb = run_bass_kernel_spmd(nc, in_maps, core_ids=list(range(NCORES)), **kw)

    total = 0.0
    for c in range(NCORES):
        v = 2.0 * rb.results[c]["res"].astype(np.float64)  # (128, 256)
        # col = ((d*NQUAD + t)*8 + i)*4 + g ; value = min dist_sq for
        # point index 128*i + part of batch BPC*c + 4*t + g, direction d
        d_sq = np.maximum(v, 0.0) + 1e-16
        total += np.sqrt(d_sq).sum()
    out = np.float32(total / 2.0)
    return out, rb


def kernel(p, q):
    out, _ = _run(p, q)
    return out

